# revision 10
# baseline (speedup 1.0000x reference)
"""Trainium2 Bass kernel for nn_NetworkRNNCell (gnn message passing).

Contract: kernel(**inputs) takes FULL unsharded numpy inputs (as produced by
setup_inputs()) and returns the FULL output tuple (out, v_new, s_new),
matching reference() exactly in shapes/dtypes.

Sharding: the unit axis N (=4096) of the synaptic weight matrices W[S,N,N]
is split column-wise across the 8 NeuronCores (512 output columns per core).
Every core processes all S=20 synapses for its column slice:
  - drive[s, j] = sum_i pre[s,i] * W[s,i,j]   (PE matmuls, i tiled by 128)
  - synapse update, conductance currents, segment-sum onto the P=4 target
    populations (one tiny 0/1 selection-matrix matmul), Euler integration,
    output rates -- all elementwise over the local j slice.
This needs no cross-core collectives: the segment-sum runs over the synapse
axis which stays fully local. The only global reduction is the scalar
stability error; each core emits 4 partial sums which the host combines
during unshard.
"""

import sys
import types

import numpy as np

import concourse.bacc as bacc
import concourse.bass as bass
import concourse.tile as tile
from concourse import mybir
from concourse.bass_utils import run_bass_kernel_spmd

# network constants
DT = 0.1
TAU_M = 10.0
TAU_S = 5.0
P = 4
S = 20
N = 4096
B = 1
IN_FREQ = 0.008

N_CORES = 8
JC = N // N_CORES          # output columns per core (512)
KT = N // 128              # contraction tiles of 128 (32)
CH = 16                    # k-tiles per W DMA chunk (16 -> 4 MB chunks)
W_BUFS = 3                 # W tile double/triple buffering

DECAY = 1.0 - DT / TAU_S   # 0.98

# "f32r": PE fast fp32 mode (full speed, slightly relaxed precision)
# "f32" : exact fp32 matmul (4x PE cycles, still near the DMA roofline)
W_MODE = "f32r"

# Set by test.py to capture an NTFF profile; LAST_EXEC_NS then holds the
# max-over-cores NEFF execution time of the last run.
TRACE = False
LAST_EXEC_NS = None
LAST_RESULTS = None

_PROGRAM_CACHE = {}


def _install_ntff_hook():
    """Provide antenv.axon_hooks (absent on this image) and register the
    NTFF profile hook exposed by the axon boot shim."""
    if "antenv.axon_hooks" not in sys.modules:
        import antenv

        mod = types.ModuleType("antenv.axon_hooks")
        holder = [None]
        mod.set_axon_ntff_profile_hook = lambda h: holder.__setitem__(0, h)
        mod.get_axon_ntff_profile_hook = lambda: holder[0]
        sys.modules["antenv.axon_hooks"] = mod
        antenv.axon_hooks = mod
    import antenv.axon_hooks as ah

    if ah.get_axon_ntff_profile_hook() is None:
        from trn_agent_boot.trn_boot import _ntff_profile_via_ctypes

        ah.set_axon_ntff_profile_hook(
            _ntff_profile_via_ctypes("/opt/axon/libaxon_pjrt.so")
        )


def _build_program(src_idx: tuple, tgt_idx: tuple, mode: str):
    """Build + bacc-compile the SPMD Bass program (identical on all cores)."""
    f32 = mybir.dt.float32
    wdt = mybir.dt.float32r if mode == "f32r" else f32

    nc = bacc.Bacc("TRN2", target_bir_lowering=False, debug=False,
                   num_devices=N_CORES)

    w_in = nc.dram_tensor("w", [128, S, KT, JC], wdt, kind="ExternalInput").ap()
    vt_in = nc.dram_tensor("vt", [128, P, KT], f32, kind="ExternalInput").ap()
    ph_in = nc.dram_tensor("ph", [128, KT], f32, kind="ExternalInput").ap()
    t_in = nc.dram_tensor("t", [1, 1], f32, kind="ExternalInput").ap()
    s_in = nc.dram_tensor("s", [S, JC], f32, kind="ExternalInput").ap()
    v_in = nc.dram_tensor("v", [P, JC], f32, kind="ExternalInput").ap()
    pv_in = nc.dram_tensor("pv", [S, JC], f32, kind="ExternalInput").ap()
    er_in = nc.dram_tensor("er", [S, 1], f32, kind="ExternalInput").ap()
    ms_in = nc.dram_tensor("ms", [S, P], f32, kind="ExternalInput").ap()

    snew_out = nc.dram_tensor("snew", [S, JC], f32, kind="ExternalOutput").ap()
    vnew_out = nc.dram_tensor("vnew", [P, JC], f32, kind="ExternalOutput").ap()
    rates_out = nc.dram_tensor("rates", [P, JC], f32, kind="ExternalOutput").ap()
    errp_out = nc.dram_tensor("errp", [P, 1], f32, kind="ExternalOutput").ap()

    with tile.TileContext(nc) as tc:
        with (
            tc.tile_pool(name="const", bufs=1) as cpool,
            tc.tile_pool(name="wpool", bufs=W_BUFS) as wpool,
            tc.tile_pool(name="stage", bufs=4) as stage_pool,
            tc.tile_pool(name="dpsum", bufs=4, space="PSUM") as dpsum,
            tc.tile_pool(name="apsum", bufs=2, space="PSUM") as apsum,
        ):
            # ---- small resident tiles -------------------------------------
            rates_sb = cpool.tile([128, P + 1, KT], wdt)   # all_rates, lhsT layout
            vt_sb = cpool.tile([128, P, KT], f32)
            ph_sb = cpool.tile([128, KT], f32)
            t_sb = cpool.tile([128, 1], f32)
            pib_sb = cpool.tile([128, 1], f32)
            pib3_sb = cpool.tile([128, 1], f32)
            sin_sb = cpool.tile([128, KT], f32)
            s1_sb = cpool.tile([128, KT], f32)
            s2_sb = cpool.tile([128, KT], f32)
            s_sb = cpool.tile([S, JC], f32)
            v_sb = cpool.tile([P, JC], f32)
            pv_sb = cpool.tile([S, JC], f32)
            er_sb = cpool.tile([S, 1], f32)
            ms_sb = cpool.tile([S, P], f32)
            snew_sb = cpool.tile([S, JC], f32)
            sdec_sb = cpool.tile([S, JC], f32)
            wterm_sb = cpool.tile([S, JC], f32)
            isyn_sb = cpool.tile([S, JC], f32)
            gv_sb = cpool.tile([P, JC], f32)
            dv_sb = cpool.tile([P, JC], f32)
            vnew_sb = cpool.tile([P, JC], f32)
            rates4_sb = cpool.tile([P, JC], f32)
            err_sb = cpool.tile([P, 1], f32)

            nc.sync.dma_start(out=vt_sb, in_=vt_in)
            nc.sync.dma_start(out=ph_sb, in_=ph_in)
            nc.sync.dma_start(out=s_sb, in_=s_in)
            nc.sync.dma_start(out=v_sb, in_=v_in)
            nc.sync.dma_start(out=pv_sb, in_=pv_in)
            nc.sync.dma_start(out=er_sb, in_=er_in)
            nc.sync.dma_start(out=ms_sb, in_=ms_in)
            # broadcast t over the 128 partitions
            nc.gpsimd.dma_start(out=t_sb, in_=t_in.to_broadcast((128, 1)))

            # ---- firing rates of all source populations -------------------
            # rows 0..P-1: sigmoid(v); row P: 0.5*(1+sin(2*pi*f*t + phase))
            nc.scalar.activation(
                out=rates_sb[:, 0:P, :], in_=vt_sb,
                func=mybir.ActivationFunctionType.Sigmoid,
            )
            # ScalarE Sin needs args in [-pi, pi]. a = phase + t_red lies in
            # [0, 4pi) (host pre-reduces the 2*pi*f*t scalar mod 2pi), so
            # subtract 2*pi*k with k = (sign(a-pi) + sign(a-3pi))/2 + 1.
            nc.vector.memset(pib_sb, -float(np.pi))
            nc.vector.memset(pib3_sb, -float(3.0 * np.pi))
            nc.vector.tensor_scalar(
                out=sin_sb, in0=ph_sb, scalar1=t_sb, scalar2=None,
                op0=mybir.AluOpType.add,
            )
            nc.scalar.activation(
                out=s1_sb, in_=sin_sb,
                func=mybir.ActivationFunctionType.Sign, bias=pib_sb,
            )
            nc.scalar.activation(
                out=s2_sb, in_=sin_sb,
                func=mybir.ActivationFunctionType.Sign, bias=pib3_sb,
            )
            nc.vector.tensor_add(s1_sb, s1_sb, s2_sb)
            nc.vector.tensor_scalar(
                out=s1_sb, in0=s1_sb,
                scalar1=-float(np.pi), scalar2=-float(2.0 * np.pi),
                op0=mybir.AluOpType.mult, op1=mybir.AluOpType.add,
            )
            nc.vector.tensor_add(sin_sb, sin_sb, s1_sb)
            nc.scalar.activation(
                out=sin_sb, in_=sin_sb,
                func=mybir.ActivationFunctionType.Sin,
            )
            nc.vector.tensor_scalar(
                out=rates_sb[:, P, :], in0=sin_sb,
                scalar1=0.5, scalar2=0.5,
                op0=mybir.AluOpType.mult, op1=mybir.AluOpType.add,
            )

            # ---- per-synapse drive matmuls + synapse update ---------------
            for s in range(S):
                r = src_idx[s]
                drive_ps = dpsum.tile([1, JC], f32, tag="drive")
                for c0 in range(0, KT, CH):
                    w_tile = wpool.tile([128, CH, JC], wdt, tag="w")
                    nc.sync.dma_start(out=w_tile, in_=w_in[:, s, c0:c0 + CH, :])
                    for k in range(CH):
                        kk = c0 + k
                        nc.tensor.matmul(
                            drive_ps, rates_sb[:, r, kk:kk + 1], w_tile[:, k, :],
                            start=(kk == 0), stop=(kk == KT - 1),
                        )
                # s_new partial: DT * drive. ACT stages PSUM -> SBUF at
                # partition 0 (engines can't start mid-partition), then a
                # tiny SBUF->SBUF DMA scatters it to row s.
                stage_sb = stage_pool.tile([1, JC], f32, tag="stage")
                nc.scalar.mul(stage_sb, drive_ps, DT)
                nc.sync.dma_start(out=snew_sb[s:s + 1, :], in_=stage_sb)

            # ---- synapse state + currents (batched over all 20 rows) ------
            nc.vector.tensor_scalar_mul(sdec_sb, s_sb, DECAY)
            nc.vector.tensor_add(snew_sb, snew_sb, sdec_sb)
            nc.sync.dma_start(out=snew_out, in_=snew_sb)
            # wterm = E_rev - post_v
            nc.vector.tensor_scalar(
                out=wterm_sb, in0=pv_sb, scalar1=-1.0, scalar2=er_sb,
                op0=mybir.AluOpType.mult, op1=mybir.AluOpType.add,
            )
            nc.vector.tensor_mul(isyn_sb, snew_sb, wterm_sb)

            # ---- segment-sum onto targets via 0/1 selection matmul --------
            itot_ps = apsum.tile([P, JC], f32, tag="acc")
            gtot_ps = apsum.tile([P, JC], f32, tag="acc")
            nc.tensor.matmul(itot_ps, ms_sb, isyn_sb, start=True, stop=True)
            nc.tensor.matmul(gtot_ps, ms_sb, snew_sb, start=True, stop=True)

            # ---- population integration -----------------------------------
            nc.vector.tensor_mul(gv_sb, gtot_ps, v_sb)
            nc.vector.tensor_sub(dv_sb, itot_ps, gv_sb)
            nc.vector.tensor_sub(dv_sb, dv_sb, v_sb)
            nc.vector.tensor_scalar_mul(dv_sb, dv_sb, 1.0 / TAU_M)
            nc.vector.tensor_reduce(
                out=err_sb, in_=dv_sb, axis=mybir.AxisListType.X,
                op=mybir.AluOpType.add, apply_absolute_value=True,
            )
            nc.vector.tensor_scalar_mul(vnew_sb, dv_sb, DT)
            nc.vector.tensor_add(vnew_sb, vnew_sb, v_sb)
            nc.scalar.activation(
                out=rates4_sb, in_=vnew_sb,
                func=mybir.ActivationFunctionType.Sigmoid,
            )

            nc.sync.dma_start(out=vnew_out, in_=vnew_sb)
            nc.sync.dma_start(out=rates_out, in_=rates4_sb)
            nc.sync.dma_start(out=errp_out, in_=err_sb)

    nc.compile()
    return nc


def _get_program(src_idx, tgt_idx, mode):
    key = (tuple(int(x) for x in src_idx), tuple(int(x) for x in tgt_idx), mode)
    if key not in _PROGRAM_CACHE:
        _PROGRAM_CACHE[key] = _build_program(key[0], key[1], mode)
    return _PROGRAM_CACHE[key]


def _prep_inputs(t, v, s, W, E_rev, phase, src_idx, tgt_idx):
    """Host-side shard/layout prep. No math beyond gathers/transposes."""
    f32 = np.float32
    v2 = np.asarray(v, f32)[:, 0, :]                      # [P, N]
    s2 = np.asarray(s, f32)[:, 0, :]                      # [S, N]
    W = np.asarray(W, f32)
    tgt = np.asarray(tgt_idx)

    # lhsT layouts: element [p, ..., k] = x[..., k*128 + p]
    vt = np.ascontiguousarray(v2.reshape(P, KT, 128).transpose(2, 0, 1))
    ph = np.ascontiguousarray(
        np.asarray(phase, f32).reshape(KT, 128).transpose(1, 0))
    msel = np.zeros((S, P), f32)
    msel[np.arange(S), tgt] = 1.0
    erev = np.asarray(E_rev, f32).reshape(S, 1)
    # t enters as the fp32 angle 2*pi*f*t, pre-reduced mod 2pi (one host
    # scalar op) so the on-device sin range reduction only spans [0, 4pi).
    tr = f32(2.0 * np.pi * IN_FREQ) * np.asarray(t, f32).reshape(-1)[0]
    if tr >= f32(2.0 * np.pi) or tr < 0.0:
        tr = f32(np.float64(tr) % (2.0 * np.pi))
    t_arr = np.asarray(tr, f32).reshape(1, 1)

    # W[s, i, j] with i = kt*128 + p, j = c*JC + jj  ->  per-core [128, S, KT, JC]
    W5 = W.reshape(S, KT, 128, N_CORES, JC)

    in_maps = []
    for c in range(N_CORES):
        wc = np.ascontiguousarray(W5[:, :, :, c, :].transpose(2, 0, 1, 3))
        sl = slice(c * JC, (c + 1) * JC)
        vc = np.ascontiguousarray(v2[:, sl])
        in_maps.append(dict(
            w=wc, vt=vt, ph=ph, t=t_arr,
            s=np.ascontiguousarray(s2[:, sl]),
            v=vc,
            pv=np.ascontiguousarray(vc[tgt]),
            er=erev, ms=msel,
        ))
    return in_maps


def kernel(t, v, s, W, E_rev, phase, src_idx, tgt_idx):
    global LAST_EXEC_NS, LAST_RESULTS
    nc = _get_program(src_idx, tgt_idx, W_MODE)
    in_maps = _prep_inputs(t, v, s, W, E_rev, phase, src_idx, tgt_idx)

    kwargs = {}
    if TRACE:
        _install_ntff_hook()
        kwargs = dict(trace=True, trace_cores=list(range(N_CORES)))
    res = run_bass_kernel_spmd(nc, in_maps, core_ids=list(range(N_CORES)),
                               **kwargs)
    LAST_EXEC_NS = res.exec_time_ns
    LAST_RESULTS = res

    # ---- unshard -----------------------------------------------------------
    f32 = np.float32
    s_new = np.empty((S, B, N), f32)
    v_new = np.empty((P, B, N), f32)
    rates = np.empty((P, N), f32)
    err_acc = 0.0
    for c in range(N_CORES):
        r = res.results[c]
        sl = slice(c * JC, (c + 1) * JC)
        s_new[:, 0, sl] = r["snew"]
        v_new[:, 0, sl] = r["vnew"]
        rates[:, sl] = r["rates"]
        err_acc += float(r["errp"].sum())
    err = np.asarray(err_acc / N, f32)
    out = np.concatenate(
        [rates.reshape(1, P * N), err.reshape(1, 1)], axis=1).astype(f32)
    return out, v_new, s_new


# revision 14
# speedup vs baseline: 1.0193x; 1.0193x over previous
"""Trainium2 Bass kernel for nn_NetworkRNNCell (gnn message passing).

Contract: kernel(**inputs) takes FULL unsharded numpy inputs (as produced by
setup_inputs()) and returns the FULL output tuple (out, v_new, s_new),
matching reference() exactly in shapes/dtypes.

Sharding: the unit axis N (=4096) of the synaptic weight matrices W[S,N,N]
is split column-wise across the 8 NeuronCores (512 output columns per core).
Every core processes all S=20 synapses for its column slice:
  - drive[s, j] = sum_i pre[s,i] * W[s,i,j]   (PE matmuls, i tiled by 128)
  - synapse update, conductance currents, segment-sum onto the P=4 target
    populations (one tiny 0/1 selection-matrix matmul), Euler integration,
    output rates -- all elementwise over the local j slice.
This needs no cross-core collectives: the segment-sum runs over the synapse
axis which stays fully local. The only global reduction is the scalar
stability error; each core emits 4 partial sums which the host combines
during unshard.
"""

import sys
import types

import numpy as np

import concourse.bacc as bacc
import concourse.bass as bass
import concourse.tile as tile
from concourse import mybir
from concourse.bass_utils import run_bass_kernel_spmd

# network constants
DT = 0.1
TAU_M = 10.0
TAU_S = 5.0
P = 4
S = 20
N = 4096
B = 1
IN_FREQ = 0.008

N_CORES = 8
JC = N // N_CORES          # output columns per core (512)
KT = N // 128              # contraction tiles of 128 (32)
CH = 8                     # k-tiles per W DMA chunk (8 -> 2 MB chunks)
W_BUFS = 6                 # W tile buffering depth

DECAY = 1.0 - DT / TAU_S   # 0.98

# "f32r": PE fast fp32 mode (full speed, slightly relaxed precision)
# "f32" : exact fp32 matmul (4x PE cycles, still near the DMA roofline)
W_MODE = "f32r"

# Set by test.py to capture an NTFF profile; LAST_EXEC_NS then holds the
# max-over-cores NEFF execution time of the last run.
TRACE = False
LAST_EXEC_NS = None
LAST_RESULTS = None

_PROGRAM_CACHE = {}


def _install_ntff_hook():
    """Provide antenv.axon_hooks (absent on this image) and register the
    NTFF profile hook exposed by the axon boot shim."""
    if "antenv.axon_hooks" not in sys.modules:
        import antenv

        mod = types.ModuleType("antenv.axon_hooks")
        holder = [None]
        mod.set_axon_ntff_profile_hook = lambda h: holder.__setitem__(0, h)
        mod.get_axon_ntff_profile_hook = lambda: holder[0]
        sys.modules["antenv.axon_hooks"] = mod
        antenv.axon_hooks = mod
    import antenv.axon_hooks as ah

    if ah.get_axon_ntff_profile_hook() is None:
        from trn_agent_boot.trn_boot import _ntff_profile_via_ctypes

        ah.set_axon_ntff_profile_hook(
            _ntff_profile_via_ctypes("/opt/axon/libaxon_pjrt.so")
        )


def _build_program(src_idx: tuple, tgt_idx: tuple, mode: str):
    """Build + bacc-compile the SPMD Bass program (identical on all cores)."""
    f32 = mybir.dt.float32
    wdt = mybir.dt.float32r if mode == "f32r" else f32

    nc = bacc.Bacc("TRN2", target_bir_lowering=False, debug=False,
                   num_devices=N_CORES)

    w_in = nc.dram_tensor("w", [128, S, KT, JC], wdt, kind="ExternalInput").ap()
    vt_in = nc.dram_tensor("vt", [128, P, KT], f32, kind="ExternalInput").ap()
    ph_in = nc.dram_tensor("ph", [128, KT], f32, kind="ExternalInput").ap()
    t_in = nc.dram_tensor("t", [1, 1], f32, kind="ExternalInput").ap()
    s_in = nc.dram_tensor("s", [S, JC], f32, kind="ExternalInput").ap()
    v_in = nc.dram_tensor("v", [P, JC], f32, kind="ExternalInput").ap()
    pv_in = nc.dram_tensor("pv", [S, JC], f32, kind="ExternalInput").ap()
    er_in = nc.dram_tensor("er", [S, 1], f32, kind="ExternalInput").ap()
    ms_in = nc.dram_tensor("ms", [S, P], f32, kind="ExternalInput").ap()

    snew_out = nc.dram_tensor("snew", [S, JC], f32, kind="ExternalOutput").ap()
    vnew_out = nc.dram_tensor("vnew", [P, JC], f32, kind="ExternalOutput").ap()
    rates_out = nc.dram_tensor("rates", [P, JC], f32, kind="ExternalOutput").ap()
    errp_out = nc.dram_tensor("errp", [P, 1], f32, kind="ExternalOutput").ap()

    with tile.TileContext(nc) as tc:
        with (
            tc.tile_pool(name="const", bufs=1) as cpool,
            tc.tile_pool(name="wpool", bufs=W_BUFS) as wpool,
            tc.tile_pool(name="stage", bufs=4) as stage_pool,
            tc.tile_pool(name="dpsum", bufs=4, space="PSUM") as dpsum,
            tc.tile_pool(name="apsum", bufs=2, space="PSUM") as apsum,
        ):
            # ---- small resident tiles -------------------------------------
            rates_sb = cpool.tile([128, P + 1, KT], wdt)   # all_rates, lhsT layout
            vt_sb = cpool.tile([128, P, KT], f32)
            ph_sb = cpool.tile([128, KT], f32)
            t_sb = cpool.tile([128, 1], f32)
            pib_sb = cpool.tile([128, 1], f32)
            pib3_sb = cpool.tile([128, 1], f32)
            sin_sb = cpool.tile([128, KT], f32)
            s1_sb = cpool.tile([128, KT], f32)
            s2_sb = cpool.tile([128, KT], f32)
            s_sb = cpool.tile([S, JC], f32)
            v_sb = cpool.tile([P, JC], f32)
            pv_sb = cpool.tile([S, JC], f32)
            er_sb = cpool.tile([S, 1], f32)
            ms_sb = cpool.tile([S, P], f32)
            snew_sb = cpool.tile([S, JC], f32)
            sdec_sb = cpool.tile([S, JC], f32)
            wterm_sb = cpool.tile([S, JC], f32)
            isyn_sb = cpool.tile([S, JC], f32)
            gv_sb = cpool.tile([P, JC], f32)
            dv_sb = cpool.tile([P, JC], f32)
            vnew_sb = cpool.tile([P, JC], f32)
            rates4_sb = cpool.tile([P, JC], f32)
            err_sb = cpool.tile([P, 1], f32)

            nc.sync.dma_start(out=vt_sb, in_=vt_in)
            nc.sync.dma_start(out=ph_sb, in_=ph_in)
            nc.sync.dma_start(out=s_sb, in_=s_in)
            nc.sync.dma_start(out=v_sb, in_=v_in)
            nc.sync.dma_start(out=pv_sb, in_=pv_in)
            nc.sync.dma_start(out=er_sb, in_=er_in)
            nc.sync.dma_start(out=ms_sb, in_=ms_in)
            # broadcast t over the 128 partitions
            nc.gpsimd.dma_start(out=t_sb, in_=t_in.to_broadcast((128, 1)))

            # ---- firing rates of all source populations -------------------
            # rows 0..P-1: sigmoid(v); row P: 0.5*(1+sin(2*pi*f*t + phase))
            nc.scalar.activation(
                out=rates_sb[:, 0:P, :], in_=vt_sb,
                func=mybir.ActivationFunctionType.Sigmoid,
            )
            # ScalarE Sin needs args in [-pi, pi]. a = phase + t_red lies in
            # [0, 4pi) (host pre-reduces the 2*pi*f*t scalar mod 2pi), so
            # subtract 2*pi*k with k = (sign(a-pi) + sign(a-3pi))/2 + 1.
            nc.vector.memset(pib_sb, -float(np.pi))
            nc.vector.memset(pib3_sb, -float(3.0 * np.pi))
            nc.vector.tensor_scalar(
                out=sin_sb, in0=ph_sb, scalar1=t_sb, scalar2=None,
                op0=mybir.AluOpType.add,
            )
            nc.scalar.activation(
                out=s1_sb, in_=sin_sb,
                func=mybir.ActivationFunctionType.Sign, bias=pib_sb,
            )
            nc.scalar.activation(
                out=s2_sb, in_=sin_sb,
                func=mybir.ActivationFunctionType.Sign, bias=pib3_sb,
            )
            nc.vector.tensor_add(s1_sb, s1_sb, s2_sb)
            nc.vector.tensor_scalar(
                out=s1_sb, in0=s1_sb,
                scalar1=-float(np.pi), scalar2=-float(2.0 * np.pi),
                op0=mybir.AluOpType.mult, op1=mybir.AluOpType.add,
            )
            nc.vector.tensor_add(sin_sb, sin_sb, s1_sb)
            nc.scalar.activation(
                out=sin_sb, in_=sin_sb,
                func=mybir.ActivationFunctionType.Sin,
            )
            nc.vector.tensor_scalar(
                out=rates_sb[:, P, :], in0=sin_sb,
                scalar1=0.5, scalar2=0.5,
                op0=mybir.AluOpType.mult, op1=mybir.AluOpType.add,
            )

            # ---- per-synapse drive matmuls + synapse update ---------------
            for s in range(S):
                r = src_idx[s]
                drive_ps = dpsum.tile([1, JC], f32, tag="drive")
                for c0 in range(0, KT, CH):
                    w_tile = wpool.tile([128, CH, JC], wdt, tag="w")
                    nc.sync.dma_start(out=w_tile, in_=w_in[:, s, c0:c0 + CH, :])
                    for k in range(CH):
                        kk = c0 + k
                        nc.tensor.matmul(
                            drive_ps, rates_sb[:, r, kk:kk + 1], w_tile[:, k, :],
                            start=(kk == 0), stop=(kk == KT - 1),
                        )
                # s_new partial: DT * drive. ACT stages PSUM -> SBUF at
                # partition 0 (engines can't start mid-partition), then a
                # tiny SBUF->SBUF DMA scatters it to row s.
                stage_sb = stage_pool.tile([1, JC], f32, tag="stage")
                nc.scalar.mul(stage_sb, drive_ps, DT)
                # SWDGE path: keeps this dependent little DMA out of the
                # HWDGE FIFOs that stream the W chunks
                nc.gpsimd.dma_start(out=snew_sb[s:s + 1, :], in_=stage_sb)

            # ---- synapse state + currents (batched over all 20 rows) ------
            nc.vector.tensor_scalar_mul(sdec_sb, s_sb, DECAY)
            nc.vector.tensor_add(snew_sb, snew_sb, sdec_sb)
            nc.sync.dma_start(out=snew_out, in_=snew_sb)
            # wterm = E_rev - post_v
            nc.vector.tensor_scalar(
                out=wterm_sb, in0=pv_sb, scalar1=-1.0, scalar2=er_sb,
                op0=mybir.AluOpType.mult, op1=mybir.AluOpType.add,
            )
            nc.vector.tensor_mul(isyn_sb, snew_sb, wterm_sb)

            # ---- segment-sum onto targets via 0/1 selection matmul --------
            itot_ps = apsum.tile([P, JC], f32, tag="acc")
            gtot_ps = apsum.tile([P, JC], f32, tag="acc")
            nc.tensor.matmul(itot_ps, ms_sb, isyn_sb, start=True, stop=True)
            nc.tensor.matmul(gtot_ps, ms_sb, snew_sb, start=True, stop=True)

            # ---- population integration -----------------------------------
            nc.vector.tensor_mul(gv_sb, gtot_ps, v_sb)
            nc.vector.tensor_sub(dv_sb, itot_ps, gv_sb)
            nc.vector.tensor_sub(dv_sb, dv_sb, v_sb)
            nc.vector.tensor_scalar_mul(dv_sb, dv_sb, 1.0 / TAU_M)
            nc.vector.tensor_reduce(
                out=err_sb, in_=dv_sb, axis=mybir.AxisListType.X,
                op=mybir.AluOpType.add, apply_absolute_value=True,
            )
            nc.vector.tensor_scalar_mul(vnew_sb, dv_sb, DT)
            nc.vector.tensor_add(vnew_sb, vnew_sb, v_sb)
            nc.scalar.activation(
                out=rates4_sb, in_=vnew_sb,
                func=mybir.ActivationFunctionType.Sigmoid,
            )

            nc.sync.dma_start(out=vnew_out, in_=vnew_sb)
            nc.sync.dma_start(out=rates_out, in_=rates4_sb)
            nc.sync.dma_start(out=errp_out, in_=err_sb)

    nc.compile()
    return nc


def _get_program(src_idx, tgt_idx, mode):
    key = (tuple(int(x) for x in src_idx), tuple(int(x) for x in tgt_idx), mode)
    if key not in _PROGRAM_CACHE:
        _PROGRAM_CACHE[key] = _build_program(key[0], key[1], mode)
    return _PROGRAM_CACHE[key]


def _prep_inputs(t, v, s, W, E_rev, phase, src_idx, tgt_idx):
    """Host-side shard/layout prep. No math beyond gathers/transposes."""
    f32 = np.float32
    v2 = np.asarray(v, f32)[:, 0, :]                      # [P, N]
    s2 = np.asarray(s, f32)[:, 0, :]                      # [S, N]
    W = np.asarray(W, f32)
    tgt = np.asarray(tgt_idx)

    # lhsT layouts: element [p, ..., k] = x[..., k*128 + p]
    vt = np.ascontiguousarray(v2.reshape(P, KT, 128).transpose(2, 0, 1))
    ph = np.ascontiguousarray(
        np.asarray(phase, f32).reshape(KT, 128).transpose(1, 0))
    msel = np.zeros((S, P), f32)
    msel[np.arange(S), tgt] = 1.0
    erev = np.asarray(E_rev, f32).reshape(S, 1)
    # t enters as the fp32 angle 2*pi*f*t, pre-reduced mod 2pi (one host
    # scalar op) so the on-device sin range reduction only spans [0, 4pi).
    tr = f32(2.0 * np.pi * IN_FREQ) * np.asarray(t, f32).reshape(-1)[0]
    if tr >= f32(2.0 * np.pi) or tr < 0.0:
        tr = f32(np.float64(tr) % (2.0 * np.pi))
    t_arr = np.asarray(tr, f32).reshape(1, 1)

    # W[s, i, j] with i = kt*128 + p, j = c*JC + jj  ->  per-core [128, S, KT, JC]
    W5 = W.reshape(S, KT, 128, N_CORES, JC)

    in_maps = []
    for c in range(N_CORES):
        wc = np.ascontiguousarray(W5[:, :, :, c, :].transpose(2, 0, 1, 3))
        sl = slice(c * JC, (c + 1) * JC)
        vc = np.ascontiguousarray(v2[:, sl])
        in_maps.append(dict(
            w=wc, vt=vt, ph=ph, t=t_arr,
            s=np.ascontiguousarray(s2[:, sl]),
            v=vc,
            pv=np.ascontiguousarray(vc[tgt]),
            er=erev, ms=msel,
        ))
    return in_maps


def kernel(t, v, s, W, E_rev, phase, src_idx, tgt_idx):
    global LAST_EXEC_NS, LAST_RESULTS
    nc = _get_program(src_idx, tgt_idx, W_MODE)
    in_maps = _prep_inputs(t, v, s, W, E_rev, phase, src_idx, tgt_idx)

    kwargs = {}
    if TRACE:
        _install_ntff_hook()
        kwargs = dict(trace=True, trace_cores=list(range(N_CORES)))
    res = run_bass_kernel_spmd(nc, in_maps, core_ids=list(range(N_CORES)),
                               **kwargs)
    LAST_EXEC_NS = res.exec_time_ns
    LAST_RESULTS = res

    # ---- unshard -----------------------------------------------------------
    f32 = np.float32
    s_new = np.empty((S, B, N), f32)
    v_new = np.empty((P, B, N), f32)
    rates = np.empty((P, N), f32)
    err_acc = 0.0
    for c in range(N_CORES):
        r = res.results[c]
        sl = slice(c * JC, (c + 1) * JC)
        s_new[:, 0, sl] = r["snew"]
        v_new[:, 0, sl] = r["vnew"]
        rates[:, sl] = r["rates"]
        err_acc += float(r["errp"].sum())
    err = np.asarray(err_acc / N, f32)
    out = np.concatenate(
        [rates.reshape(1, P * N), err.reshape(1, 1)], axis=1).astype(f32)
    return out, v_new, s_new
